# revision 6
# baseline (speedup 1.0000x reference)
"""ClassConditionalLM log-likelihood kernel for 8 Trainium2 NeuronCores.

Range-plane weight-stationary formulation:
  out[n] = logsumexp_j( x[n, j] ),
  x[n,j] = prior''_j - sum_l maskf[l,n]*stab[l,j] + sum_l d(v[l,n], j+1)*w[l,j]

One-hot via telescoped range masks: d(v, j+1) = R_{j+1} - R_{j+2} with
R_s = [v >= s] (R_65 = 0). Reorganized per plane s = 1..64:
  x[n,:] += sum_l R_s[l,n] * V_s[l,:]
where V_s has two nonzero columns (+w[:,s-1] at class s-1, -w[:,s-2] at
class s-2), and V_1 additionally carries the dense -stab (maskf == R_1,
hi/lo bf16 split since stab errors are correlated across the 128 LFs).

Device mapping (per core, data-parallel over instances):
  - Planes R_s [L=128, F] are built by DVE (is_ge, bf16 4x), ACT (Sign ->
    +-1 flavor, algebra rescaled, constants folded into prior''), and
    GPSIMD (is_ge), statically scheduled to balance engine finish times.
  - The PE consumes each plane as the STATIONARY operand (Ldweights) per
    128-instance tile and streams the tiny 2-column rhs V_s, accumulating
    x directly in PSUM [128 inst, 64 classes] per tile -- ~6ns per matmul,
    so the PE is far off the critical path (vs streaming masks at 853ns
    per plane). prior'' is injected by a ones-lhsT matmul (hi/lo bf16).
    PSUM zero regions are 2KB/partition: start=True only on the first
    matmul per bank (t % 8 == 0).
  - Tail per chunk: one batched DVE max-reduce (negated), per-tile ACT Exp
    with accumulated row-sum. The previous chunk's max-reduce/exps are
    emitted AFTER the current chunk's plane builds so DVE/ACT never stall
    at chunk boundaries. All Ln's once at the end + single output DMA.
"""

import math

import numpy as np
import ml_dtypes

N, L, K = 131072, 128, 64
M = 8                    # NeuronCores
NC_N = N // M            # 16384 instances per core
FMAX = 2048              # largest chunk (PSUM/vote tile allocation size)
LOGKM1 = math.log(K - 1)

# chunk sizes: small first chunk fills the pipeline fast (short first DMA),
# small last chunks shrink the serial drain (ttr+exps of the final chunk).
CHUNKS = [1024] + [2048] * 7 + [1024]
assert sum(CHUNKS) == NC_N

# plane->engine counts (dve, act, pool), from the measured rates
# 1127/3598/5784 ns per [128,4096] plane: LP optimum ~ (44.3, 10.7, 9.0).
# The act set must be static across chunks (sign flavor is baked into the
# host-side rhs tables and prior folds).
_ENG_COST = {"dve": 594.0, "act": 1850.0, "pool": 2939.0}

# per-chunk (dve, act, pool) plane counts; all planes use the +-1/2 "sign"
# algebra so any plane can be built on any engine (DVE/Pool emit +-0.5 via
# fused is_ge,subtract; ACT emits +-1 Sign and consumes half-scaled rhs).
CHUNK_ENG_N = [
    {"dve": 42, "act": 13, "pool": 9},
    {"dve": 45, "act": 11, "pool": 8},
    {"dve": 42, "act": 12, "pool": 10},
    {"dve": 44, "act": 11, "pool": 9},
    {"dve": 44, "act": 11, "pool": 9},
    {"dve": 43, "act": 11, "pool": 10},
    {"dve": 44, "act": 11, "pool": 9},
    {"dve": 45, "act": 10, "pool": 9},
    {"dve": 46, "act": 7, "pool": 11},
]


def _plane_schedule(eng_n):
    slots = []
    for eng, n in eng_n.items():
        for k in range(1, n + 1):
            slots.append((k * _ENG_COST[eng], eng))
    slots.sort()
    return [eng for _, eng in slots]          # index s-1 -> engine


CHUNK_SCHED = [_plane_schedule(en) for en in CHUNK_ENG_N]
for pe_ in CHUNK_SCHED:
    assert len(pe_) == K

_BASS_CACHE: dict = {}


def _build_bass(nc_n: int):
    import concourse.mybir as mybir
    from concourse.bacc import Bacc
    from concourse.tile import TileContext

    dt = mybir.dt
    Alu = mybir.AluOpType
    Act = mybir.ActivationFunctionType

    assert sum(CHUNKS) == nc_n
    ncols = nc_n // 128          # total 128-instance column tiles

    nc = Bacc()
    votest = nc.dram_tensor("votest", [L, nc_n], dt.bfloat16, kind="ExternalInput")
    v1ht = nc.dram_tensor("v1ht", [L, 2 * K], dt.bfloat16, kind="ExternalInput")
    v1lt = nc.dram_tensor("v1lt", [L, 2 * K], dt.bfloat16, kind="ExternalInput")
    w0t = nc.dram_tensor("w0t", [L, 2], dt.bfloat16, kind="ExternalInput")
    ptab = nc.dram_tensor("ptab", [L, 2 * (K - 1) * 2], dt.bfloat16,
                          kind="ExternalInput")
    pht = nc.dram_tensor("pht", [L, K], dt.bfloat16, kind="ExternalInput")
    plt = nc.dram_tensor("plt", [L, K], dt.bfloat16, kind="ExternalInput")
    abias = nc.dram_tensor("abias", [128, K], dt.float32, kind="ExternalInput")
    # out[p, x] = loglik of instance x*128 + p; host transposes (a strided
    # device DMA into instance order costs ~10us of descriptor processing)
    out = nc.dram_tensor("out", [128, nc_n // 128], dt.float32,
                         kind="ExternalOutput")

    with TileContext(nc) as tc:
        with (
            tc.tile_pool(name="const", bufs=1) as cpool,
            tc.tile_pool(name="vt", bufs=4) as vpool,
            tc.tile_pool(name="plane", bufs=32) as mpool,
            tc.tile_pool(name="escr", bufs=4) as epool,
            tc.tile_pool(name="pc", bufs=4, space="PSUM") as pcpool,
        ):
            # chunk-0 votes first in the DMA queue (DVE starts earliest),
            # then abias (ACT sign planes), then the PE-side tables.
            vts = [
                vpool.tile([L, FMAX], dt.bfloat16, tag="vt", name=f"vt{i}")
                for i in range(4)
            ]
            nc.sync.dma_start(out=vts[0][:, 0:CHUNKS[0]], in_=votest[:, 0:CHUNKS[0]])
            abias_sb = cpool.tile([128, K], dt.float32, tag="abias")
            nc.sync.dma_start(out=abias_sb[:], in_=abias[:, :])
            ph_sb = cpool.tile([L, K], dt.bfloat16, tag="ph")
            nc.sync.dma_start(out=ph_sb[:], in_=pht[:, :])
            pl_sb = cpool.tile([L, K], dt.bfloat16, tag="pl")
            nc.sync.dma_start(out=pl_sb[:], in_=plt[:, :])
            v1h_sb = cpool.tile([L, 2 * K], dt.bfloat16, tag="v1h")
            nc.sync.dma_start(out=v1h_sb[:], in_=v1ht[:, :])
            v1l_sb = cpool.tile([L, 2 * K], dt.bfloat16, tag="v1l")
            nc.sync.dma_start(out=v1l_sb[:], in_=v1lt[:, :])
            w0_sb = cpool.tile([L, 2], dt.bfloat16, tag="w0")
            nc.sync.dma_start(out=w0_sb[:], in_=w0t[:, :])
            ptab_sb = cpool.tile([L, 2 * (K - 1) * 2], dt.bfloat16, tag="ptab")
            nc.sync.dma_start(out=ptab_sb[:], in_=ptab[:, :])
            ones_sb = cpool.tile([L, 128], dt.bfloat16, tag="ones")
            nc.vector.memset(ones_sb[:], 1.0)
            # warm the ACT function table immediately (the auto-inserted
            # LoadActFuncSet lands before this dep-free dummy instead of
            # stalling behind the first sign plane's DMA dependencies)
            warm = cpool.tile([128, 2], dt.float32, tag="warm")
            nc.vector.memset(warm[:], 0.0)
            nc.scalar.activation(out=warm[:, 1:2], in_=warm[:, 0:1],
                                 func=Act.Exp, bias=warm[:, 0:1], scale=1.0)
            # per-column-tile logsumexp pieces, stashed until the end
            ssum_all = cpool.tile([128, ncols], dt.float32, tag="ssum_all")
            mneg_all = cpool.tile([128, ncols], dt.float32, tag="mneg_all")

            prev = None
            pend_ttr = None      # (pc, colo, tpt) whose max-reduce is pending
            off = 0              # running instance offset
            colo = 0             # running 128-col tile offset
            for c, F in enumerate(CHUNKS):
                TPT = F // 128
                if c == 0:
                    vt = vts[0]
                else:
                    vt = vts[c % 4]
                    nc.sync.dma_start(out=vt[:, 0:F],
                                      in_=votest[:, off:off + F])

                pc = pcpool.tile([128, FMAX // 128 * K], dt.float32, tag="pc")

                last_chunk = c == len(CHUNKS) - 1
                if last_chunk:
                    for t in range(TPT):
                        po = pc[:, t * K:(t + 1) * K]
                        nc.tensor.matmul(
                            out=po, lhsT=ones_sb[:], rhs=ph_sb[:],
                            start=(t % 8 == 0), stop=False,
                            skip_group_check=True,
                        )
                        nc.tensor.matmul(
                            out=po, lhsT=ones_sb[:], rhs=pl_sb[:],
                            start=False, stop=False, skip_group_check=True,
                        )

                sched = CHUNK_SCHED[c]
                ndve = 0
                for s in range(1, K + 1):
                    eng = sched[s - 1]
                    if eng == "dve":
                        ndve += 1
                        if ndve == 3 and pend_ttr is not None:
                            # previous chunk's max-reduce: DVE has primed the
                            # PE pipeline, so this wait costs no DVE work
                            _emit_ttr(nc, mybir, *pend_ttr, mneg_all)
                            pend_ttr = None
                    pl_t = mpool.tile([L, FMAX], dt.bfloat16, tag="plane")
                    if eng == "act":
                        # +-1 plane; consumed with the half-scaled rhs (h=1)
                        nc.scalar.activation(
                            out=pl_t[:, 0:F], in_=vt[:, 0:F], func=Act.Sign,
                            bias=abias_sb[:, s - 1:s], scale=1.0,
                        )
                        h = 1
                    else:
                        # +-0.5 plane via fused (is_ge, subtract); full rhs
                        meng = nc.gpsimd if eng == "pool" else nc.vector
                        meng.tensor_scalar(
                            out=pl_t[:, 0:F], in0=vt[:, 0:F], scalar1=float(s),
                            scalar2=0.5, op0=Alu.is_ge, op1=Alu.subtract,
                        )
                        h = 0
                    for t in range(TPT):
                        lh = pl_t[:, t * 128:(t + 1) * 128]
                        if s == 1:
                            # plane 1 runs first on the PE and carries the
                            # start=True bank init (2KB zero region per bank)
                            nc.tensor.matmul(
                                out=pc[:, t * K:(t + 1) * K], lhsT=lh,
                                rhs=v1h_sb[:, h * K:(h + 1) * K],
                                start=(t % 8 == 0) and not last_chunk,
                                stop=False, skip_group_check=True,
                            )
                            nc.tensor.matmul(
                                out=pc[:, t * K:(t + 1) * K], lhsT=lh,
                                rhs=v1l_sb[:, h * K:(h + 1) * K],
                                start=False, stop=False,
                                skip_group_check=True,
                            )
                            nc.tensor.matmul(
                                out=pc[:, t * K:t * K + 1], lhsT=lh,
                                rhs=w0_sb[:, h:h + 1], start=False, stop=False,
                                skip_group_check=True,
                            )
                        else:
                            base = h * (K - 1) * 2 + (s - 2) * 2
                            nc.tensor.matmul(
                                out=pc[:, t * K + s - 2:t * K + s], lhsT=lh,
                                rhs=ptab_sb[:, base:base + 2],
                                start=False, stop=False,
                                skip_group_check=True,
                            )

                # prior'' via ones-lhsT matmuls (hi/lo), at the END of the
                # PE stream: their PSUM-buffer wait (previous tenant's exps)
                # resolves long before the PE drains the planes above, so
                # the PE starts each window immediately on plane 1.
                for t in range(0 if last_chunk else TPT):
                    po = pc[:, t * K:(t + 1) * K]
                    nc.tensor.matmul(
                        out=po, lhsT=ones_sb[:], rhs=ph_sb[:],
                        start=False, stop=False, skip_group_check=True,
                    )
                    nc.tensor.matmul(
                        out=po, lhsT=ones_sb[:], rhs=pl_sb[:],
                        start=False, stop=(t % 8 == 7), skip_group_check=True,
                    )

                # this chunk's max-reduce is deferred into the NEXT chunk's
                # DVE stream (else DVE stalls on the PE drain); the exps are
                # deferred into the NEXT chunk's ACT stream likewise.
                if prev is not None:
                    _emit_exps(nc, mybir, *prev, mneg_all, ssum_all, epool)
                pend_ttr = (pc, colo, TPT)
                prev = (pc, colo, TPT)
                off += F
                colo += TPT

            _emit_ttr(nc, mybir, *pend_ttr, mneg_all)
            _emit_exps(nc, mybir, *prev, mneg_all, ssum_all, epool)

            # finale: ln over all stashed sums, add back maxes, single DMA out
            lns = cpool.tile([128, ncols], dt.float32, tag="lns")
            nc.scalar.activation(out=lns[:], in_=ssum_all[:], func=Act.Ln)
            outT = cpool.tile([128, ncols], dt.float32, tag="outT")
            nc.vector.tensor_tensor(
                out=outT[:], in0=lns[:], in1=mneg_all[:], op=Alu.subtract,
            )
            nc.sync.dma_start(out=out[:, :], in_=outT[:])
    nc.finalize()
    return nc


def _emit_ttr(nc, mybir, pc, colo, tpt, mneg_all):
    Alu = mybir.AluOpType
    nc.vector.tensor_reduce(
        out=mneg_all[:, colo:colo + tpt],
        in_=pc[:, 0:tpt * K].rearrange("p (t k) -> p t k", k=K),
        axis=mybir.AxisListType.X, op=Alu.max, negate=True,
    )


def _emit_exps(nc, mybir, pc, colo, tpt, mneg_all, ssum_all, epool):
    dt = mybir.dt
    Act = mybir.ActivationFunctionType
    for t in range(tpt):
        col = colo + t
        escr = epool.tile([128, K], dt.float32, tag="escr")
        nc.scalar.activation(
            out=escr[:], in_=pc[:, t * K:(t + 1) * K], func=Act.Exp,
            bias=mneg_all[:, col:col + 1], scale=1.0,
            accum_out=ssum_all[:, col:col + 1],
        )


def _get_bass(nc_n: int):
    if nc_n not in _BASS_CACHE:
        _BASS_CACHE[nc_n] = _build_bass(nc_n)
    return _BASS_CACHE[nc_n]


def _prepare_host(votes, accuracy, propensity, class_balance):
    bf16 = ml_dtypes.bfloat16
    votes = np.asarray(votes)
    accuracy = np.asarray(accuracy, dtype=np.float32)
    propensity = np.asarray(propensity, dtype=np.float32)
    class_balance = np.asarray(class_balance, dtype=np.float32)

    # values 0..64 are exact in bf16
    votesT = np.ascontiguousarray(votes.T.astype(np.float32).astype(bf16))

    z_acc = np.logaddexp(accuracy, -accuracy)
    stab = (z_acc + accuracy - propensity[:, None] + LOGKM1).astype(np.float32)
    w = (2.0 * accuracy + LOGKM1).astype(np.float32)
    wb = w.astype(bf16).astype(np.float32)       # single bf16 rounding of w

    zprop = np.logaddexp(propensity, 0.0)
    cbm = class_balance.max()
    cb = class_balance - (np.log(np.sum(np.exp(class_balance - cbm))) + cbm)
    prior = (cb - zprop.sum()).astype(np.float64)  # [K]; folds added below

    # Every plane uses the "sign" algebra: contribution = sum_l q*V_full
    # with q = R - 1/2 (DVE/Pool fused op) or (Q=+-1) * V_full/2 (ACT),
    # plus a constant fold of  1/2 * sum_l V_full  into the prior.
    # Full-scale tables (h=0) and exactly-halved tables (h=1) side by side.

    # plane 1: V1 = -stab in hi/lo bf16 (+w0 via separate column matmul)
    v1f = (-stab).astype(np.float64)
    v1h0 = v1f.astype(np.float32).astype(bf16)
    v1l0 = (v1f - v1h0.astype(np.float64)).astype(np.float32).astype(bf16)
    w00 = wb[:, 0:1].astype(bf16)
    prior += 0.5 * (v1h0.astype(np.float64).sum(0)
                    + v1l0.astype(np.float64).sum(0))
    prior[0] += 0.5 * w00.astype(np.float64).sum()

    # planes 2..64: two columns each
    ptab0 = np.zeros((L, K - 1, 2), np.float32)
    for s in range(2, K + 1):
        ptab0[:, s - 2, 0] = -wb[:, s - 2]
        ptab0[:, s - 2, 1] = wb[:, s - 1]
    ptab0 = ptab0.astype(bf16)
    for s in range(2, K + 1):
        prior[s - 2] += 0.5 * ptab0[:, s - 2, 0].astype(np.float64).sum()
        prior[s - 1] += 0.5 * ptab0[:, s - 2, 1].astype(np.float64).sum()

    def _half(a):
        return (a.astype(np.float32) * 0.5).astype(bf16)   # exact in bf16

    v1h = np.ascontiguousarray(np.concatenate([v1h0, _half(v1h0)], axis=1))
    v1l = np.ascontiguousarray(np.concatenate([v1l0, _half(v1l0)], axis=1))
    w0 = np.ascontiguousarray(np.concatenate([w00, _half(w00)], axis=1))
    p0 = ptab0.reshape(L, (K - 1) * 2)
    ptab = np.ascontiguousarray(np.concatenate([p0, _half(p0)], axis=1))

    pr = (prior / 128.0).astype(np.float32)
    ph = pr.astype(bf16)
    pl = (pr - ph.astype(np.float32)).astype(bf16)
    ph = np.ascontiguousarray(np.broadcast_to(ph[None, :], (L, K)))
    pl = np.ascontiguousarray(np.broadcast_to(pl[None, :], (L, K)))

    abias = np.ascontiguousarray(
        np.broadcast_to(
            (0.5 - np.arange(1, K + 1, dtype=np.float32))[None, :], (128, K)
        )
    ).astype(np.float32)

    return votesT, v1h, v1l, w0, ptab, ph, pl, abias


def _run(votes, accuracy, propensity, class_balance, trace=False):
    from concourse.bass_utils import run_bass_kernel_spmd

    votesT, v1h, v1l, w0, ptab, ph, pl, abias = _prepare_host(
        votes, accuracy, propensity, class_balance
    )
    nc = _get_bass(NC_N)
    in_maps = []
    for c in range(M):
        in_maps.append({
            "votest": np.ascontiguousarray(votesT[:, c * NC_N:(c + 1) * NC_N]),
            "v1ht": v1h,
            "v1lt": v1l,
            "w0t": w0,
            "ptab": ptab,
            "pht": ph,
            "plt": pl,
            "abias": abias,
        })
    res = run_bass_kernel_spmd(
        nc, in_maps, core_ids=list(range(M)), trace=trace
    )
    # device leaves out as [128, nc_n//128] with instance = x*128 + p
    out = np.concatenate([
        np.asarray(r["out"]).T.reshape(-1) for r in res.results
    ])
    return out.astype(np.float32), res


def kernel(votes, accuracy, propensity, class_balance):
    out, _ = _run(votes, accuracy, propensity, class_balance)
    return out


def kernel_with_stats(votes, accuracy, propensity, class_balance):
    try:
        out, res = _run(votes, accuracy, propensity, class_balance, trace=True)
    except (ImportError, ModuleNotFoundError):
        # no NTFF profiling hook in this environment; run without trace
        out, res = _run(votes, accuracy, propensity, class_balance, trace=False)
    return out, res


def simulate_ns() -> float:
    """Cost-model timeline estimate (ns) of one core's NEFF execution."""
    from concourse.timeline_sim import TimelineSim

    return TimelineSim(_get_bass(NC_N), trace=False).simulate()
